# revision 1
# baseline (speedup 1.0000x reference)
"""Trainium2 Bass kernel for nn_PreampGainLayer.

The reference computes, per batch row b:
    st = concat(state[b, L:], x[b])                    # length N=4096
    out[b] = irfft(rfft(st) * h_b)[-L:]                # circular filter
where h_b = num_b(w)/den_b(w) evaluated at w = e^{-i pi k/2048} is a biquad
transfer function whose denominator has two REAL roots w1, w2 with |w| < 1
(RC circuit, bilinear transform).  Partial fractions give

    H_b(w) = n0_b + r1_b/(w - w1_b) + r2_b/(w - w2_b)

and the circular convolution with each 1/(w - wi) kernel is an anti-causal
one-pole IIR: z_i[m] = sum_{s>=0} wi^s st[(m+1+s) mod N]  (wi^N ~ 1e-38 -> 0).

Device algorithm per batch (batch rows on SBUF partitions):
    head-scan (backward over state half):  h0 = sum_j wi^j st[j]
    tail-scan (backward over x half, initial=h0): exact circular z_i
    y = n0*st[L:] + r1*z1 + r2*z2

Scans map 1:1 onto the DVE `tensor_tensor_scan` instruction (state =
data0*state + data1 along the free dim) with reversed access patterns.
The O(B) scalar parameter derivation (2x2 algebra + root finding) runs on
host in float64; all per-sample work runs on the NeuronCores, data-parallel
over batch (8 cores x 256 rows).
"""

import numpy as np

import concourse.bass as bass
import concourse.bacc as bacc
import concourse.tile as tile
import concourse.mybir as mybir
from concourse.bass_utils import run_bass_kernel_spmd

N_CORES = 8
B = 2048
L = 2048  # x length == output length
N = 4096  # filter length (state length)
BPC = B // N_CORES          # batches per core (256)
NTILES = BPC // 128         # partition tiles per core (2)

SR = 44100
RG0, R10, C10, C20 = 1.0e6, 4.7e5, 3.3e-9, 1.0e-9

_Nu = np.array([[1.0, 0.0, 0.0]])
_Nx = np.array([[1.0, -1.0, 0.0], [0.0, 0.0, 1.0]])
_Nr = np.array([[0.0, 1.0, 0.0]])
_Nv = np.array([[0.0, 1.0, -1.0], [0.0, 0.0, 1.0]])
_No = np.array([[0.0, 0.0, 1.0]])

TRACE = False           # set by test harness to capture an NTFF profile
LAST_RESULT = None      # BassKernelResults of the most recent run


def _sigmoid(x):
    return 1.0 / (1.0 + np.exp(-x))


def _derive_filter_params(cond, alpha_rg, alpha_r1, alpha_c1, alpha_c2,
                          cond_w, cond_b):
    """Float64 host derivation of per-batch (w1, w2, r1, r2, n0)."""
    T = 1.0 / SR
    cond = np.asarray(cond, np.float64)
    Bn = cond.shape[0]

    RG = (0.9 + _sigmoid(float(alpha_rg[0])) * 0.2) * RG0
    R1 = (0.99 + _sigmoid(float(alpha_r1[0])) * 0.02) * R10
    C1 = (0.9 + _sigmoid(float(alpha_c1[0])) * 0.2) * C10
    C2 = (0.9 + _sigmoid(float(alpha_c2[0])) * 0.2) * C20

    Gr = np.array([[1.0 / R1]])
    Gx = np.diag([2.0 * C1 / T, 2.0 * C2 / T])

    Nvp = np.concatenate([_Nv, np.zeros((2, 1))], axis=1)
    Nxp = np.concatenate([_Nx, np.zeros((2, 1))], axis=1)
    Nop = np.concatenate([_No, np.zeros((1, 1))], axis=1)
    Nup = np.concatenate([np.zeros((3, 1)), np.eye(1)], axis=0)

    top = np.concatenate([_Nr.T @ Gr @ _Nr + _Nx.T @ Gx @ _Nx, _Nu.T], axis=1)
    bot = np.concatenate([_Nu, np.zeros((1, 1))], axis=1)
    So_inv = np.linalg.inv(np.concatenate([top, bot], axis=0))

    Q = Nvp @ So_inv @ Nvp.T
    Ux = Nxp @ So_inv @ Nvp.T
    Uo = Nop @ So_inv @ Nvp.T
    Uu = Nup.T @ So_inv @ Nvp.T
    ZGx = 2.0 * Gx
    Ao = ZGx @ Nxp @ So_inv @ Nxp.T - np.eye(2)
    Bo = ZGx @ Nxp @ So_inv @ Nup
    Do = Nop @ So_inv @ Nxp.T
    Eo = Nop @ So_inv @ Nup
    ZGxUx = ZGx @ Ux

    pot = _sigmoid(cond[:, 0] * float(cond_w[0]) + float(cond_b[0]))
    p = np.clip((np.power(10.0, pot) - 1.0) / 9.0, 1e-4, 1.0 - 1e-4)

    M00 = (1.0 - p) * RG + Q[0, 0]
    M01 = np.full(Bn, Q[0, 1])
    M10 = np.full(Bn, Q[1, 0])
    M11 = p * RG + Q[1, 1]
    detM = M00 * M11 - M01 * M10
    I00, I01 = M11 / detM, -M01 / detM
    I10, I11 = -M10 / detM, M00 / detM

    def sandwich(Lm, Rm):
        out = np.empty((Bn, Lm.shape[0], Rm.shape[1]))
        for i in range(Lm.shape[0]):
            for j in range(Rm.shape[1]):
                out[:, i, j] = (
                    Lm[i, 0] * (I00 * Rm[0, j] + I01 * Rm[1, j])
                    + Lm[i, 1] * (I10 * Rm[0, j] + I11 * Rm[1, j])
                )
        return out

    A = Ao[None] - sandwich(ZGxUx, Ux.T)
    Bm = Bo[None] - sandwich(ZGxUx, Uu.T)
    Dm = Do[None] - sandwich(Uo, Ux.T)
    Em = Eo[None] - sandwich(Uo, Uu.T)

    tr = A[:, 0, 0] + A[:, 1, 1]
    det = A[:, 0, 0] * A[:, 1, 1] - A[:, 0, 1] * A[:, 1, 0]
    dd1, dd2 = -tr, det

    M2 = A - Bm @ Dm
    tr2 = M2[:, 0, 0] + M2[:, 1, 1]
    det2 = M2[:, 0, 0] * M2[:, 1, 1] - M2[:, 0, 1] * M2[:, 1, 0]
    e = Em[:, 0, 0] - 1.0
    n0 = 1.0 + e
    n1 = -tr2 + e * dd1
    n2 = det2 + e * dd2

    a = n1 - n0 * dd1
    b = n2 - n0 * dd2
    disc = dd1 * dd1 - 4.0 * dd2
    if np.any(disc <= 0):
        raise ValueError("complex poles: real-pole fast path invalid")
    sq = np.sqrt(disc)
    w1 = 0.5 * (-dd1 + sq)
    w2 = 0.5 * (-dd1 - sq)
    r1 = (a * w1 + b) / (w1 - w2)
    r2 = (a * w2 + b) / (w2 - w1)
    return w1, w2, r1, r2, n0


def _rev(ap):
    """Reverse the innermost free dim of an AP (unit-stride dims only)."""
    step, cnt = ap.ap[-1]
    assert step == 1, ap.ap
    return bass.AP(
        tensor=ap.tensor,
        offset=ap.offset + cnt - 1,
        ap=[list(p) for p in ap.ap[:-1]] + [[-1, cnt]],
    )


def _build_nc():
    f32 = mybir.dt.float32
    mult, add = mybir.AluOpType.mult, mybir.AluOpType.add

    nc = bacc.Bacc("TRN2", target_bir_lowering=False, debug=False)
    x_in = nc.dram_tensor("x", [BPC, L], f32, kind="ExternalInput")
    s_in = nc.dram_tensor("state_tail", [BPC, L], f32, kind="ExternalInput")
    # columns: w1, w2, r1, r2, n0
    p_in = nc.dram_tensor("fparams", [BPC, 8], f32, kind="ExternalInput")
    y_out = nc.dram_tensor("y", [BPC, L], f32, kind="ExternalOutput")

    with tile.TileContext(nc) as tc:
        with (
            tc.tile_pool(name="sig", bufs=2) as sig,
            tc.tile_pool(name="par", bufs=2) as parp,
            tc.tile_pool(name="z", bufs=4) as zp,
            tc.tile_pool(name="y", bufs=4) as yp,
        ):
            for t in range(NTILES):
                rows = slice(t * 128, (t + 1) * 128)
                st = sig.tile([128, N], f32)
                par = parp.tile([128, 8], f32)
                nc.sync.dma_start(par[:, :], p_in[rows, :])
                nc.sync.dma_start(st[:, 0:L], s_in[rows, :])
                nc.sync.dma_start(st[:, L:N], x_in[rows, :])

                zs = []
                for pole in range(2):
                    wcol = par[:, pole : pole + 1]
                    hscr = zp.tile([128, L], f32)
                    # h0 = sum_{j<L} w^j st[j]: backward scan over state half
                    nc.vector.tensor_tensor_scan(
                        _rev(hscr[:, :]),
                        wcol.broadcast_to([128, L]),
                        _rev(st[:, 0:L]),
                        0.0, mult, add,
                    )
                    z = zp.tile([128, L], f32)
                    # z[m] = w z[m+1] + st[m+1], m = N-2 .. L, init z[N-1]=h0
                    nc.vector.tensor_tensor_scan(
                        _rev(z[:, 0 : L - 1]),
                        wcol.broadcast_to([128, L - 1]),
                        _rev(st[:, L + 1 : N]),
                        hscr[:, 0:1], mult, add,
                    )
                    nc.scalar.copy(z[:, L - 1 : L], hscr[:, 0:1])
                    zs.append(z)

                y0 = yp.tile([128, L], f32)
                nc.scalar.mul(y0[:, :], st[:, L:N], par[:, 4:5])  # n0 * x
                y1 = yp.tile([128, L], f32)
                nc.vector.scalar_tensor_tensor(
                    y1[:, :], zs[0][:, :], par[:, 2:3], y0[:, :], mult, add
                )
                y2 = yp.tile([128, L], f32)
                nc.vector.scalar_tensor_tensor(
                    y2[:, :], zs[1][:, :], par[:, 3:4], y1[:, :], mult, add
                )
                nc.sync.dma_start(y_out[rows, :], y2[:, :])

    nc.compile()
    return nc


_NC_CACHE = None


def _get_nc():
    global _NC_CACHE
    if _NC_CACHE is None:
        _NC_CACHE = _build_nc()
    return _NC_CACHE


def kernel(x, cond, state, alpha_rg, alpha_r1, alpha_c1, alpha_c2,
           cond_w, cond_b):
    global LAST_RESULT
    x = np.ascontiguousarray(np.asarray(x, np.float32)[:, :, 0])          # [B, L]
    state_tail = np.ascontiguousarray(
        np.asarray(state, np.float32)[:, L:, 0])                          # [B, L]

    w1, w2, r1, r2, n0 = _derive_filter_params(
        np.asarray(cond), np.asarray(alpha_rg), np.asarray(alpha_r1),
        np.asarray(alpha_c1), np.asarray(alpha_c2), np.asarray(cond_w),
        np.asarray(cond_b),
    )
    fparams = np.zeros((B, 8), np.float32)
    fparams[:, 0] = w1
    fparams[:, 1] = w2
    fparams[:, 2] = r1
    fparams[:, 3] = r2
    fparams[:, 4] = n0

    nc = _get_nc()
    in_maps = []
    for c in range(N_CORES):
        rows = slice(c * BPC, (c + 1) * BPC)
        in_maps.append({
            "x": x[rows],
            "state_tail": state_tail[rows],
            "fparams": fparams[rows],
        })

    res = run_bass_kernel_spmd(
        nc, in_maps, core_ids=list(range(N_CORES)), trace=TRACE
    )
    LAST_RESULT = res
    out = np.concatenate([r["y"] for r in res.results], axis=0)
    return out.reshape(B, L, 1).astype(np.float32)


# revision 2
# speedup vs baseline: 1.0893x; 1.0893x over previous
"""Trainium2 Bass kernel for nn_PreampGainLayer.

The reference computes, per batch row b:
    st = concat(state[b, L:], x[b])                    # length N=4096
    out[b] = irfft(rfft(st) * h_b)[-L:]                # circular filter
where h_b = num_b(w)/den_b(w) evaluated at w = e^{-i pi k/2048} is a biquad
transfer function whose denominator has two REAL roots w1, w2 with |w| < 1
(RC circuit, bilinear transform).  Partial fractions give

    H_b(w) = n0_b + r1_b/(w - w1_b) + r2_b/(w - w2_b)

and the circular convolution with each 1/(w - wi) kernel is an anti-causal
one-pole IIR: z_i[m] = sum_{s>=0} wi^s st[(m+1+s) mod N]  (wi^N ~ 1e-38 -> 0).

Device algorithm per batch (batch rows on SBUF partitions):
    h0_i = sum_{j<1024} wi^j st[j]          (wi^1024 < 2e-9: tail truncated)
         = accum_out of one fused multiply, weights wi^j = Exp(j*ln wi)
           built on ScalarE
    tail-scan (backward over x half, initial=h0_i): exact circular z_i
           via the DVE tensor_tensor_scan recurrence state=w*state+data
    y = n0*x + r1*z1 + r2*z2                (scalar_tensor_tensor FMAs)

The O(B) scalar parameter derivation (2x2 algebra + root finding) runs on
host in float64; all per-sample work runs on the NeuronCores, data-parallel
over batch (8 cores x 256 rows).
"""

import numpy as np

import concourse.bass as bass
import concourse.bacc as bacc
import concourse.tile as tile
import concourse.mybir as mybir
from concourse.bass_utils import run_bass_kernel_spmd

N_CORES = 8
B = 2048
L = 2048          # x length == output length
HEAD = 1024       # retained prefix of the state half
BPC = B // N_CORES
NTILES = BPC // 128
HL = L // 2       # half of the output, for pipelining chunks

SR = 44100
RG0, R10, C10, C20 = 1.0e6, 4.7e5, 3.3e-9, 1.0e-9

_Nu = np.array([[1.0, 0.0, 0.0]])
_Nx = np.array([[1.0, -1.0, 0.0], [0.0, 0.0, 1.0]])
_Nr = np.array([[0.0, 1.0, 0.0]])
_Nv = np.array([[0.0, 1.0, -1.0], [0.0, 0.0, 1.0]])
_No = np.array([[0.0, 0.0, 1.0]])

TRACE = False
LAST_RESULT = None


def _sigmoid(x):
    return 1.0 / (1.0 + np.exp(-x))


def _derive_filter_params(cond, alpha_rg, alpha_r1, alpha_c1, alpha_c2,
                          cond_w, cond_b):
    """Float64 host derivation of per-batch (w1, w2, r1, r2, n0)."""
    T = 1.0 / SR
    cond = np.asarray(cond, np.float64)
    Bn = cond.shape[0]

    RG = (0.9 + _sigmoid(float(alpha_rg[0])) * 0.2) * RG0
    R1 = (0.99 + _sigmoid(float(alpha_r1[0])) * 0.02) * R10
    C1 = (0.9 + _sigmoid(float(alpha_c1[0])) * 0.2) * C10
    C2 = (0.9 + _sigmoid(float(alpha_c2[0])) * 0.2) * C20

    Gr = np.array([[1.0 / R1]])
    Gx = np.diag([2.0 * C1 / T, 2.0 * C2 / T])

    Nvp = np.concatenate([_Nv, np.zeros((2, 1))], axis=1)
    Nxp = np.concatenate([_Nx, np.zeros((2, 1))], axis=1)
    Nop = np.concatenate([_No, np.zeros((1, 1))], axis=1)
    Nup = np.concatenate([np.zeros((3, 1)), np.eye(1)], axis=0)

    top = np.concatenate([_Nr.T @ Gr @ _Nr + _Nx.T @ Gx @ _Nx, _Nu.T], axis=1)
    bot = np.concatenate([_Nu, np.zeros((1, 1))], axis=1)
    So_inv = np.linalg.inv(np.concatenate([top, bot], axis=0))

    Q = Nvp @ So_inv @ Nvp.T
    Ux = Nxp @ So_inv @ Nvp.T
    Uo = Nop @ So_inv @ Nvp.T
    Uu = Nup.T @ So_inv @ Nvp.T
    ZGx = 2.0 * Gx
    Ao = ZGx @ Nxp @ So_inv @ Nxp.T - np.eye(2)
    Bo = ZGx @ Nxp @ So_inv @ Nup
    Do = Nop @ So_inv @ Nxp.T
    Eo = Nop @ So_inv @ Nup
    ZGxUx = ZGx @ Ux

    pot = _sigmoid(cond[:, 0] * float(cond_w[0]) + float(cond_b[0]))
    p = np.clip((np.power(10.0, pot) - 1.0) / 9.0, 1e-4, 1.0 - 1e-4)

    M00 = (1.0 - p) * RG + Q[0, 0]
    M01 = np.full(Bn, Q[0, 1])
    M10 = np.full(Bn, Q[1, 0])
    M11 = p * RG + Q[1, 1]
    detM = M00 * M11 - M01 * M10
    I00, I01 = M11 / detM, -M01 / detM
    I10, I11 = -M10 / detM, M00 / detM

    def sandwich(Lm, Rm):
        out = np.empty((Bn, Lm.shape[0], Rm.shape[1]))
        for i in range(Lm.shape[0]):
            for j in range(Rm.shape[1]):
                out[:, i, j] = (
                    Lm[i, 0] * (I00 * Rm[0, j] + I01 * Rm[1, j])
                    + Lm[i, 1] * (I10 * Rm[0, j] + I11 * Rm[1, j])
                )
        return out

    A = Ao[None] - sandwich(ZGxUx, Ux.T)
    Bm = Bo[None] - sandwich(ZGxUx, Uu.T)
    Dm = Do[None] - sandwich(Uo, Ux.T)
    Em = Eo[None] - sandwich(Uo, Uu.T)

    tr = A[:, 0, 0] + A[:, 1, 1]
    det = A[:, 0, 0] * A[:, 1, 1] - A[:, 0, 1] * A[:, 1, 0]
    dd1, dd2 = -tr, det

    M2 = A - Bm @ Dm
    tr2 = M2[:, 0, 0] + M2[:, 1, 1]
    det2 = M2[:, 0, 0] * M2[:, 1, 1] - M2[:, 0, 1] * M2[:, 1, 0]
    e = Em[:, 0, 0] - 1.0
    n0 = 1.0 + e
    n1 = -tr2 + e * dd1
    n2 = det2 + e * dd2

    a = n1 - n0 * dd1
    b = n2 - n0 * dd2
    disc = dd1 * dd1 - 4.0 * dd2
    if np.any(disc <= 0):
        raise ValueError("complex poles: real-pole fast path invalid")
    sq = np.sqrt(disc)
    w1 = 0.5 * (-dd1 + sq)
    w2 = 0.5 * (-dd1 - sq)
    r1 = (a * w1 + b) / (w1 - w2)
    r2 = (a * w2 + b) / (w2 - w1)
    return w1, w2, r1, r2, n0


def _rev(ap):
    """Reverse the innermost free dim of an AP (unit-stride dims only)."""
    step, cnt = ap.ap[-1]
    assert step == 1, ap.ap
    return bass.AP(
        tensor=ap.tensor,
        offset=ap.offset + cnt - 1,
        ap=[list(p) for p in ap.ap[:-1]] + [[-1, cnt]],
    )


def _build_nc():
    f32 = mybir.dt.float32
    mult, add = mybir.AluOpType.mult, mybir.AluOpType.add
    bypass = mybir.AluOpType.bypass

    nc = bacc.Bacc("TRN2", target_bir_lowering=False, debug=False)
    x_in = nc.dram_tensor("x", [BPC, L], f32, kind="ExternalInput")
    s_in = nc.dram_tensor("state_head", [BPC, HEAD], f32, kind="ExternalInput")
    # columns: w1, w2, r1, r2, n0, lnw1, lnw2, pad
    p_in = nc.dram_tensor("fparams", [BPC, 8], f32, kind="ExternalInput")
    y_out = nc.dram_tensor("y", [BPC, L], f32, kind="ExternalOutput")

    with tile.TileContext(nc) as tc:
        with (
            tc.tile_pool(name="sig", bufs=2) as sig,
            tc.tile_pool(name="par", bufs=2) as parp,
            tc.tile_pool(name="wgt", bufs=4) as wp,
            tc.tile_pool(name="scr", bufs=2) as scrp,
            tc.tile_pool(name="h0", bufs=4) as h0p,
            tc.tile_pool(name="z", bufs=4) as zp,
            tc.tile_pool(name="y", bufs=8) as yp,
            tc.tile_pool(name="const", bufs=1) as cst,
        ):
            iota = cst.tile([128, HEAD], f32)
            nc.gpsimd.iota(iota[:, :], pattern=[[1, HEAD]], base=0,
                           channel_multiplier=0,
                           allow_small_or_imprecise_dtypes=True)

            for t in range(NTILES):
                rows = slice(t * 128, (t + 1) * 128)
                # st layout: cols [0, HEAD) = state head, [HEAD, HEAD+L) = x
                st = sig.tile([128, HEAD + L], f32)
                par = parp.tile([128, 8], f32)
                nc.sync.dma_start(par[:, :], p_in[rows, :])
                nc.sync.dma_start(st[:, 0:HEAD], s_in[rows, :])
                # x high half first: the backward tail scan consumes it first
                nc.sync.dma_start(st[:, HEAD + HL : HEAD + L],
                                  x_in[rows, HL:L])
                nc.sync.dma_start(st[:, HEAD : HEAD + HL], x_in[rows, 0:HL])

                zs = []
                h0s = []
                for pole in range(2):
                    wcol = par[:, pole : pole + 1]
                    lncol = par[:, 5 + pole : 6 + pole]
                    wtile = wp.tile([128, HEAD], f32)
                    # w^j = exp(j * ln w)
                    nc.scalar.activation(wtile[:, :], iota[:, :],
                                         mybir.ActivationFunctionType.Exp,
                                         scale=lncol)
                    scr = scrp.tile([128, HEAD], f32)
                    h0 = h0p.tile([128, 1], f32)
                    # h0 = sum_j w^j st[j]
                    nc.vector.scalar_tensor_tensor(
                        scr[:, :], st[:, 0:HEAD], 1.0, wtile[:, :],
                        bypass, mult, accum_out=h0[:, :],
                    )
                    h0s.append(h0)

                    # z[m] = w z[m+1] + x[m+1], m = L-2 .. 0 (z[L-1] = h0),
                    # split in two chained chunks for DMA overlap.
                    z = zp.tile([128, L], f32)
                    nc.vector.tensor_tensor_scan(
                        _rev(z[:, HL : L - 1]),
                        wcol.broadcast_to([128, HL - 1]),
                        _rev(st[:, HEAD + HL + 1 : HEAD + L]),
                        h0[:, :], mult, add,
                    )
                    nc.vector.tensor_tensor_scan(
                        _rev(z[:, 0:HL]),
                        wcol.broadcast_to([128, HL]),
                        _rev(st[:, HEAD + 1 : HEAD + HL + 1]),
                        z[:, HL : HL + 1], mult, add,
                    )
                    nc.scalar.copy(z[:, L - 1 : L], h0[:, :])
                    zs.append(z)

                # y = n0*x + r1*z1 + r2*z2, assembled per half for overlap
                for h in range(2):
                    cs = slice(h * HL, (h + 1) * HL)
                    xs = slice(HEAD + h * HL, HEAD + (h + 1) * HL)
                    y0 = yp.tile([128, HL], f32)
                    nc.scalar.mul(y0[:, :], st[:, xs], par[:, 4:5])
                    y1 = yp.tile([128, HL], f32)
                    nc.vector.scalar_tensor_tensor(
                        y1[:, :], zs[0][:, cs], par[:, 2:3], y0[:, :],
                        mult, add,
                    )
                    y2 = yp.tile([128, HL], f32)
                    nc.vector.scalar_tensor_tensor(
                        y2[:, :], zs[1][:, cs], par[:, 3:4], y1[:, :],
                        mult, add,
                    )
                    nc.sync.dma_start(y_out[rows, cs], y2[:, :])

    nc.compile()
    return nc


_NC_CACHE = None


def _get_nc():
    global _NC_CACHE
    if _NC_CACHE is None:
        _NC_CACHE = _build_nc()
    return _NC_CACHE


def kernel(x, cond, state, alpha_rg, alpha_r1, alpha_c1, alpha_c2,
           cond_w, cond_b):
    global LAST_RESULT
    x = np.ascontiguousarray(np.asarray(x, np.float32)[:, :, 0])      # [B, L]
    state_head = np.ascontiguousarray(
        np.asarray(state, np.float32)[:, L : L + HEAD, 0])            # [B, HEAD]

    w1, w2, r1, r2, n0 = _derive_filter_params(
        np.asarray(cond), np.asarray(alpha_rg), np.asarray(alpha_r1),
        np.asarray(alpha_c1), np.asarray(alpha_c2), np.asarray(cond_w),
        np.asarray(cond_b),
    )
    fparams = np.zeros((B, 8), np.float32)
    fparams[:, 0] = w1
    fparams[:, 1] = w2
    fparams[:, 2] = r1
    fparams[:, 3] = r2
    fparams[:, 4] = n0
    fparams[:, 5] = np.log(w1)
    fparams[:, 6] = np.log(w2)

    nc = _get_nc()
    in_maps = []
    for c in range(N_CORES):
        rows = slice(c * BPC, (c + 1) * BPC)
        in_maps.append({
            "x": x[rows],
            "state_head": state_head[rows],
            "fparams": fparams[rows],
        })

    res = run_bass_kernel_spmd(
        nc, in_maps, core_ids=list(range(N_CORES)), trace=TRACE
    )
    LAST_RESULT = res
    out = np.concatenate([r["y"] for r in res.results], axis=0)
    return out.reshape(B, L, 1).astype(np.float32)


# revision 4
# speedup vs baseline: 1.2698x; 1.1657x over previous
"""Trainium2 Bass kernel for nn_PreampGainLayer.

The reference computes, per batch row b:
    st = concat(state[b, L:], x[b])                    # length N=4096
    out[b] = irfft(rfft(st) * h_b)[-L:]                # circular filter
where h_b = num_b(w)/den_b(w) evaluated at w = e^{-i pi k/2048} is a biquad
transfer function whose denominator has two REAL roots w1, w2 with |w| < 1
(RC circuit, bilinear transform).  Partial fractions give

    H_b(w) = n0_b + r1_b/(w - w1_b) + r2_b/(w - w2_b)

and the circular convolution with each 1/(w - wi) kernel is an anti-causal
one-pole IIR: z_i[m] = sum_{s>=0} wi^s st[(m+1+s) mod N]  (wi^N ~ 1e-38 -> 0).

Device algorithm per batch (batch rows on SBUF partitions):
    h0_i = sum_{j<1024} wi^j st[j]          (wi^1024 < 2e-9: tail truncated)
         = accum_out of one fused multiply, weights wi^j = Exp(j*ln wi)
           built on ScalarE
    tail-scan (backward over x half, initial=h0_i): exact circular z_i
           via the DVE tensor_tensor_scan recurrence state=w*state+data
    y = n0*x + r1*z1 + r2*z2                (scalar_tensor_tensor FMAs)

The O(B) scalar parameter derivation (2x2 algebra + root finding) runs on
host in float64; all per-sample work runs on the NeuronCores, data-parallel
over batch (8 cores x 256 rows).
"""

import numpy as np

import concourse.bass as bass
import concourse.bacc as bacc
import concourse.tile as tile
import concourse.mybir as mybir
from concourse.bass_utils import run_bass_kernel_spmd

N_CORES = 8
B = 2048
L = 2048          # x length == output length
HEAD = 768        # retained prefix of the state half (w^768 < 3e-7)
BPC = B // N_CORES
NTILES = BPC // 128
HL = L // 2       # half of the output, for pipelining chunks

SR = 44100
RG0, R10, C10, C20 = 1.0e6, 4.7e5, 3.3e-9, 1.0e-9

_Nu = np.array([[1.0, 0.0, 0.0]])
_Nx = np.array([[1.0, -1.0, 0.0], [0.0, 0.0, 1.0]])
_Nr = np.array([[0.0, 1.0, 0.0]])
_Nv = np.array([[0.0, 1.0, -1.0], [0.0, 0.0, 1.0]])
_No = np.array([[0.0, 0.0, 1.0]])

TRACE = False
LAST_RESULT = None


def _sigmoid(x):
    return 1.0 / (1.0 + np.exp(-x))


def _derive_filter_params(cond, alpha_rg, alpha_r1, alpha_c1, alpha_c2,
                          cond_w, cond_b):
    """Float64 host derivation of per-batch (w1, w2, r1, r2, n0)."""
    T = 1.0 / SR
    cond = np.asarray(cond, np.float64)
    Bn = cond.shape[0]

    RG = (0.9 + _sigmoid(float(alpha_rg[0])) * 0.2) * RG0
    R1 = (0.99 + _sigmoid(float(alpha_r1[0])) * 0.02) * R10
    C1 = (0.9 + _sigmoid(float(alpha_c1[0])) * 0.2) * C10
    C2 = (0.9 + _sigmoid(float(alpha_c2[0])) * 0.2) * C20

    Gr = np.array([[1.0 / R1]])
    Gx = np.diag([2.0 * C1 / T, 2.0 * C2 / T])

    Nvp = np.concatenate([_Nv, np.zeros((2, 1))], axis=1)
    Nxp = np.concatenate([_Nx, np.zeros((2, 1))], axis=1)
    Nop = np.concatenate([_No, np.zeros((1, 1))], axis=1)
    Nup = np.concatenate([np.zeros((3, 1)), np.eye(1)], axis=0)

    top = np.concatenate([_Nr.T @ Gr @ _Nr + _Nx.T @ Gx @ _Nx, _Nu.T], axis=1)
    bot = np.concatenate([_Nu, np.zeros((1, 1))], axis=1)
    So_inv = np.linalg.inv(np.concatenate([top, bot], axis=0))

    Q = Nvp @ So_inv @ Nvp.T
    Ux = Nxp @ So_inv @ Nvp.T
    Uo = Nop @ So_inv @ Nvp.T
    Uu = Nup.T @ So_inv @ Nvp.T
    ZGx = 2.0 * Gx
    Ao = ZGx @ Nxp @ So_inv @ Nxp.T - np.eye(2)
    Bo = ZGx @ Nxp @ So_inv @ Nup
    Do = Nop @ So_inv @ Nxp.T
    Eo = Nop @ So_inv @ Nup
    ZGxUx = ZGx @ Ux

    pot = _sigmoid(cond[:, 0] * float(cond_w[0]) + float(cond_b[0]))
    p = np.clip((np.power(10.0, pot) - 1.0) / 9.0, 1e-4, 1.0 - 1e-4)

    M00 = (1.0 - p) * RG + Q[0, 0]
    M01 = np.full(Bn, Q[0, 1])
    M10 = np.full(Bn, Q[1, 0])
    M11 = p * RG + Q[1, 1]
    detM = M00 * M11 - M01 * M10
    I00, I01 = M11 / detM, -M01 / detM
    I10, I11 = -M10 / detM, M00 / detM

    def sandwich(Lm, Rm):
        out = np.empty((Bn, Lm.shape[0], Rm.shape[1]))
        for i in range(Lm.shape[0]):
            for j in range(Rm.shape[1]):
                out[:, i, j] = (
                    Lm[i, 0] * (I00 * Rm[0, j] + I01 * Rm[1, j])
                    + Lm[i, 1] * (I10 * Rm[0, j] + I11 * Rm[1, j])
                )
        return out

    A = Ao[None] - sandwich(ZGxUx, Ux.T)
    Bm = Bo[None] - sandwich(ZGxUx, Uu.T)
    Dm = Do[None] - sandwich(Uo, Ux.T)
    Em = Eo[None] - sandwich(Uo, Uu.T)

    tr = A[:, 0, 0] + A[:, 1, 1]
    det = A[:, 0, 0] * A[:, 1, 1] - A[:, 0, 1] * A[:, 1, 0]
    dd1, dd2 = -tr, det

    M2 = A - Bm @ Dm
    tr2 = M2[:, 0, 0] + M2[:, 1, 1]
    det2 = M2[:, 0, 0] * M2[:, 1, 1] - M2[:, 0, 1] * M2[:, 1, 0]
    e = Em[:, 0, 0] - 1.0
    n0 = 1.0 + e
    n1 = -tr2 + e * dd1
    n2 = det2 + e * dd2

    a = n1 - n0 * dd1
    b = n2 - n0 * dd2
    disc = dd1 * dd1 - 4.0 * dd2
    if np.any(disc <= 0):
        raise ValueError("complex poles: real-pole fast path invalid")
    sq = np.sqrt(disc)
    w1 = 0.5 * (-dd1 + sq)
    w2 = 0.5 * (-dd1 - sq)
    r1 = (a * w1 + b) / (w1 - w2)
    r2 = (a * w2 + b) / (w2 - w1)
    return w1, w2, r1, r2, n0


def _rev(ap):
    """Reverse the innermost free dim of an AP (unit-stride dims only)."""
    step, cnt = ap.ap[-1]
    assert step == 1, ap.ap
    return bass.AP(
        tensor=ap.tensor,
        offset=ap.offset + cnt - 1,
        ap=[list(p) for p in ap.ap[:-1]] + [[-1, cnt]],
    )


def _build_nc():
    f32 = mybir.dt.float32
    mult, add = mybir.AluOpType.mult, mybir.AluOpType.add
    bypass = mybir.AluOpType.bypass

    nc = bacc.Bacc("TRN2", target_bir_lowering=False, debug=False)
    x_in = nc.dram_tensor("x", [BPC, L], f32, kind="ExternalInput")
    s_in = nc.dram_tensor("state_head", [BPC, HEAD], f32, kind="ExternalInput")
    # columns: w1, w2, r1, r2, n0, lnw1, lnw2, pad
    p_in = nc.dram_tensor("fparams", [BPC, 8], f32, kind="ExternalInput")
    y_out = nc.dram_tensor("y", [BPC, L], f32, kind="ExternalOutput")

    with tile.TileContext(nc) as tc:
        with (
            tc.tile_pool(name="sig", bufs=2) as sig,
            tc.tile_pool(name="par", bufs=2) as parp,
            tc.tile_pool(name="wgt", bufs=4) as wp,
            tc.tile_pool(name="scr", bufs=2) as scrp,
            tc.tile_pool(name="h0", bufs=4) as h0p,
            tc.tile_pool(name="z", bufs=4) as zp,
            tc.tile_pool(name="dg", bufs=6) as dgp,
            tc.tile_pool(name="y", bufs=4) as yp,
            tc.tile_pool(name="ps", bufs=4, space="PSUM") as psp,
            tc.tile_pool(name="const", bufs=1) as cst,
        ):
            iota = cst.tile([128, HEAD], f32)
            nc.gpsimd.iota(iota[:, :], pattern=[[1, HEAD]], base=0,
                           channel_multiplier=0,
                           allow_small_or_imprecise_dtypes=True)

            for t in range(NTILES):
                rows = slice(t * 128, (t + 1) * 128)
                # st layout: cols [0, HEAD) = state head, [HEAD, HEAD+L) = x
                st = sig.tile([128, HEAD + L], f32)
                par = parp.tile([128, 8], f32)
                nc.sync.dma_start(par[:, :], p_in[rows, :])
                nc.sync.dma_start(st[:, 0:HEAD], s_in[rows, :])
                nc.sync.dma_start(st[:, HEAD : HEAD + L], x_in[rows, :])

                # diag(n0), diag(r1), diag(r2) for the TensorE assembly
                diags = []
                for i in (4, 2, 3):
                    D = dgp.tile([128, 128], f32)
                    nc.gpsimd.affine_select(
                        D[:, :],
                        par[:, i : i + 1].broadcast_to([128, 128]),
                        pattern=[[1, 128]],
                        compare_op=mybir.AluOpType.is_equal,
                        fill=0.0, base=0, channel_multiplier=-1,
                    )
                    diags.append(D)

                zs = []
                for pole in range(2):
                    wcol = par[:, pole : pole + 1]
                    lncol = par[:, 5 + pole : 6 + pole]
                    wtile = wp.tile([128, HEAD], f32)
                    # w^j = exp(j * ln w)
                    nc.scalar.activation(wtile[:, :], iota[:, :],
                                         mybir.ActivationFunctionType.Exp,
                                         scale=lncol)
                    scr = scrp.tile([128, HEAD], f32)
                    h0 = h0p.tile([128, 1], f32)
                    # h0 = sum_j w^j st[j]
                    nc.vector.scalar_tensor_tensor(
                        scr[:, :], st[:, 0:HEAD], 1.0, wtile[:, :],
                        bypass, mult, accum_out=h0[:, :],
                    )
                    # z[m] = w z[m+1] + x[m+1], m = L-2 .. 0;  z[L-1] = h0
                    z = zp.tile([128, L], f32)
                    nc.vector.tensor_tensor_scan(
                        _rev(z[:, 0 : L - 1]),
                        wcol.broadcast_to([128, L - 1]),
                        _rev(st[:, HEAD + 1 : HEAD + L]),
                        h0[:, :], mult, add,
                    )
                    nc.scalar.copy(z[:, L - 1 : L], h0[:, :])
                    zs.append(z)

                # y = n0*x + r1*z1 + r2*z2 on TensorE via diag matmuls,
                # accumulated in PSUM; ScalarE copies out; per-1024 chunks.
                for h in range(2):
                    acc = psp.tile([128, HL], f32)
                    for q in range(2):
                        cs = slice(h * HL + q * 512, h * HL + (q + 1) * 512)
                        xs = slice(HEAD + h * HL + q * 512,
                                   HEAD + h * HL + (q + 1) * 512)
                        qs = slice(q * 512, (q + 1) * 512)
                        nc.tensor.matmul(acc[:, qs], diags[0][:, :],
                                         st[:, xs], start=True, stop=False)
                        nc.tensor.matmul(acc[:, qs], diags[1][:, :],
                                         zs[0][:, cs], start=False, stop=False)
                        nc.tensor.matmul(acc[:, qs], diags[2][:, :],
                                         zs[1][:, cs], start=False, stop=True)
                    ysb = yp.tile([128, HL], f32)
                    nc.scalar.copy(ysb[:, :], acc[:, :])
                    nc.sync.dma_start(
                        y_out[rows, h * HL : (h + 1) * HL], ysb[:, :])

    nc.compile()
    return nc


_NC_CACHE = None


def _get_nc():
    global _NC_CACHE
    if _NC_CACHE is None:
        _NC_CACHE = _build_nc()
    return _NC_CACHE


def kernel(x, cond, state, alpha_rg, alpha_r1, alpha_c1, alpha_c2,
           cond_w, cond_b):
    global LAST_RESULT
    x = np.ascontiguousarray(np.asarray(x, np.float32)[:, :, 0])      # [B, L]
    state_head = np.ascontiguousarray(
        np.asarray(state, np.float32)[:, L : L + HEAD, 0])            # [B, HEAD]

    w1, w2, r1, r2, n0 = _derive_filter_params(
        np.asarray(cond), np.asarray(alpha_rg), np.asarray(alpha_r1),
        np.asarray(alpha_c1), np.asarray(alpha_c2), np.asarray(cond_w),
        np.asarray(cond_b),
    )
    fparams = np.zeros((B, 8), np.float32)
    fparams[:, 0] = w1
    fparams[:, 1] = w2
    fparams[:, 2] = r1
    fparams[:, 3] = r2
    fparams[:, 4] = n0
    fparams[:, 5] = np.log(w1)
    fparams[:, 6] = np.log(w2)

    nc = _get_nc()
    in_maps = []
    for c in range(N_CORES):
        rows = slice(c * BPC, (c + 1) * BPC)
        in_maps.append({
            "x": x[rows],
            "state_head": state_head[rows],
            "fparams": fparams[rows],
        })

    res = run_bass_kernel_spmd(
        nc, in_maps, core_ids=list(range(N_CORES)), trace=TRACE
    )
    LAST_RESULT = res
    out = np.concatenate([r["y"] for r in res.results], axis=0)
    return out.reshape(B, L, 1).astype(np.float32)


# revision 9
# speedup vs baseline: 1.3967x; 1.1000x over previous
"""Trainium2 Bass kernel for nn_PreampGainLayer.

The reference computes, per batch row b:
    st = concat(state[b, L:], x[b])                    # length N=4096
    out[b] = irfft(rfft(st) * h_b)[-L:]                # circular filter
where h_b = num_b(w)/den_b(w) evaluated at w = e^{-i pi k/2048} is a biquad
transfer function whose denominator has two REAL roots w1, w2 with |w| < 1
(RC circuit, bilinear transform).  Partial fractions give

    H_b(w) = n0_b + r1_b/(w - w1_b) + r2_b/(w - w2_b)

and the circular convolution with each 1/(w - wi) kernel is an anti-causal
one-pole IIR: z_i[m] = sum_{s>=0} wi^s st[(m+1+s) mod N]  (wi^N ~ 1e-38 -> 0).

Device algorithm per batch (batch rows on SBUF partitions):
    h0_i = sum_{j<1024} wi^j st[j]          (wi^1024 < 2e-9: tail truncated)
         = accum_out of one fused multiply, weights wi^j = Exp(j*ln wi)
           built on ScalarE
    tail-scan (backward over x half, initial=h0_i): exact circular z_i
           via the DVE tensor_tensor_scan recurrence state=w*state+data
    y = n0*x + r1*z1 + r2*z2                (scalar_tensor_tensor FMAs)

The O(B) scalar parameter derivation (2x2 algebra + root finding) runs on
host in float64; all per-sample work runs on the NeuronCores, data-parallel
over batch (8 cores x 256 rows).
"""

import numpy as np

import concourse.bass as bass
import concourse.bacc as bacc
import concourse.tile as tile
import concourse.mybir as mybir
from concourse.bass_utils import run_bass_kernel_spmd

N_CORES = 8
B = 2048
L = 2048          # x length == output length
HEAD = 768        # retained prefix of the state half (w^768 < 3e-7)
BPC = B // N_CORES
NTILES = BPC // 128
HL = L // 2       # half of the output, for pipelining chunks

SR = 44100
RG0, R10, C10, C20 = 1.0e6, 4.7e5, 3.3e-9, 1.0e-9

_Nu = np.array([[1.0, 0.0, 0.0]])
_Nx = np.array([[1.0, -1.0, 0.0], [0.0, 0.0, 1.0]])
_Nr = np.array([[0.0, 1.0, 0.0]])
_Nv = np.array([[0.0, 1.0, -1.0], [0.0, 0.0, 1.0]])
_No = np.array([[0.0, 0.0, 1.0]])

TRACE = False
LAST_RESULT = None


def _sigmoid(x):
    return 1.0 / (1.0 + np.exp(-x))


def _derive_filter_params(cond, alpha_rg, alpha_r1, alpha_c1, alpha_c2,
                          cond_w, cond_b):
    """Float64 host derivation of per-batch (w1, w2, r1, r2, n0)."""
    T = 1.0 / SR
    cond = np.asarray(cond, np.float64)
    Bn = cond.shape[0]

    RG = (0.9 + _sigmoid(float(alpha_rg[0])) * 0.2) * RG0
    R1 = (0.99 + _sigmoid(float(alpha_r1[0])) * 0.02) * R10
    C1 = (0.9 + _sigmoid(float(alpha_c1[0])) * 0.2) * C10
    C2 = (0.9 + _sigmoid(float(alpha_c2[0])) * 0.2) * C20

    Gr = np.array([[1.0 / R1]])
    Gx = np.diag([2.0 * C1 / T, 2.0 * C2 / T])

    Nvp = np.concatenate([_Nv, np.zeros((2, 1))], axis=1)
    Nxp = np.concatenate([_Nx, np.zeros((2, 1))], axis=1)
    Nop = np.concatenate([_No, np.zeros((1, 1))], axis=1)
    Nup = np.concatenate([np.zeros((3, 1)), np.eye(1)], axis=0)

    top = np.concatenate([_Nr.T @ Gr @ _Nr + _Nx.T @ Gx @ _Nx, _Nu.T], axis=1)
    bot = np.concatenate([_Nu, np.zeros((1, 1))], axis=1)
    So_inv = np.linalg.inv(np.concatenate([top, bot], axis=0))

    Q = Nvp @ So_inv @ Nvp.T
    Ux = Nxp @ So_inv @ Nvp.T
    Uo = Nop @ So_inv @ Nvp.T
    Uu = Nup.T @ So_inv @ Nvp.T
    ZGx = 2.0 * Gx
    Ao = ZGx @ Nxp @ So_inv @ Nxp.T - np.eye(2)
    Bo = ZGx @ Nxp @ So_inv @ Nup
    Do = Nop @ So_inv @ Nxp.T
    Eo = Nop @ So_inv @ Nup
    ZGxUx = ZGx @ Ux

    pot = _sigmoid(cond[:, 0] * float(cond_w[0]) + float(cond_b[0]))
    p = np.clip((np.power(10.0, pot) - 1.0) / 9.0, 1e-4, 1.0 - 1e-4)

    M00 = (1.0 - p) * RG + Q[0, 0]
    M01 = np.full(Bn, Q[0, 1])
    M10 = np.full(Bn, Q[1, 0])
    M11 = p * RG + Q[1, 1]
    detM = M00 * M11 - M01 * M10
    I00, I01 = M11 / detM, -M01 / detM
    I10, I11 = -M10 / detM, M00 / detM

    def sandwich(Lm, Rm):
        out = np.empty((Bn, Lm.shape[0], Rm.shape[1]))
        for i in range(Lm.shape[0]):
            for j in range(Rm.shape[1]):
                out[:, i, j] = (
                    Lm[i, 0] * (I00 * Rm[0, j] + I01 * Rm[1, j])
                    + Lm[i, 1] * (I10 * Rm[0, j] + I11 * Rm[1, j])
                )
        return out

    A = Ao[None] - sandwich(ZGxUx, Ux.T)
    Bm = Bo[None] - sandwich(ZGxUx, Uu.T)
    Dm = Do[None] - sandwich(Uo, Ux.T)
    Em = Eo[None] - sandwich(Uo, Uu.T)

    tr = A[:, 0, 0] + A[:, 1, 1]
    det = A[:, 0, 0] * A[:, 1, 1] - A[:, 0, 1] * A[:, 1, 0]
    dd1, dd2 = -tr, det

    M2 = A - Bm @ Dm
    tr2 = M2[:, 0, 0] + M2[:, 1, 1]
    det2 = M2[:, 0, 0] * M2[:, 1, 1] - M2[:, 0, 1] * M2[:, 1, 0]
    e = Em[:, 0, 0] - 1.0
    n0 = 1.0 + e
    n1 = -tr2 + e * dd1
    n2 = det2 + e * dd2

    a = n1 - n0 * dd1
    b = n2 - n0 * dd2
    disc = dd1 * dd1 - 4.0 * dd2
    if np.any(disc <= 0):
        raise ValueError("complex poles: real-pole fast path invalid")
    sq = np.sqrt(disc)
    w1 = 0.5 * (-dd1 + sq)
    w2 = 0.5 * (-dd1 - sq)
    r1 = (a * w1 + b) / (w1 - w2)
    r2 = (a * w2 + b) / (w2 - w1)
    return w1, w2, r1, r2, n0


def _rev(ap):
    """Reverse the innermost free dim of an AP (unit-stride dims only)."""
    step, cnt = ap.ap[-1]
    assert step == 1, ap.ap
    return bass.AP(
        tensor=ap.tensor,
        offset=ap.offset + cnt - 1,
        ap=[list(p) for p in ap.ap[:-1]] + [[-1, cnt]],
    )


def _build_nc():
    f32 = mybir.dt.float32
    mult, add = mybir.AluOpType.mult, mybir.AluOpType.add
    bypass = mybir.AluOpType.bypass

    nc = bacc.Bacc("TRN2", target_bir_lowering=False, debug=False)
    x_in = nc.dram_tensor("x", [BPC, L], f32, kind="ExternalInput")
    s_in = nc.dram_tensor("state_head", [BPC, HEAD], f32, kind="ExternalInput")
    # columns: w1, w2, r1, r2, n0, lnw1, lnw2, pad
    p_in = nc.dram_tensor("fparams", [BPC, 8], f32, kind="ExternalInput")
    y_out = nc.dram_tensor("y", [BPC, L], f32, kind="ExternalOutput")

    with tile.TileContext(nc) as tc:
        with (
            tc.tile_pool(name="sig", bufs=2) as sig,
            tc.tile_pool(name="par", bufs=2) as parp,
            tc.tile_pool(name="wgt", bufs=4) as wp,
            tc.tile_pool(name="scr", bufs=2) as scrp,
            tc.tile_pool(name="h0", bufs=4) as h0p,
            tc.tile_pool(name="z", bufs=4) as zp,
            tc.tile_pool(name="dg", bufs=6) as dgp,
            tc.tile_pool(name="y", bufs=4) as yp,
            tc.tile_pool(name="ps", bufs=2, space="PSUM") as psp,
            tc.tile_pool(name="const", bufs=1) as cst,
        ):
            iota = cst.tile([128, HEAD], f32)
            nc.gpsimd.iota(iota[:, :], pattern=[[1, HEAD]], base=0,
                           channel_multiplier=0,
                           allow_small_or_imprecise_dtypes=True)
            # Warm the ACT exp table immediately so the real Exp calls
            # don't pay the ~1.3us ACT_TABLE_LOAD on the critical path.
            warm = cst.tile([128, 1], f32)
            nc.gpsimd.memset(warm[:, :], 0.0)
            nc.scalar.activation(warm[:, :], warm[:, :],
                                 mybir.ActivationFunctionType.Exp)

            for t in range(NTILES):
                rows = slice(t * 128, (t + 1) * 128)
                # st layout: cols [0, HEAD) = state head, [HEAD, HEAD+L) = x
                st = sig.tile([128, HEAD + L], f32)
                par = parp.tile([128, 8], f32)
                nc.sync.dma_start(par[:, :], p_in[rows, :])
                nc.sync.dma_start(st[:, 0:HEAD], s_in[rows, :])
                nc.sync.dma_start(st[:, HEAD : HEAD + L], x_in[rows, :])

                # diag(n0), diag(r1), diag(r2) for the TensorE assembly
                diags = []
                for i in (4, 2, 3):
                    D = dgp.tile([128, 128], f32)
                    nc.gpsimd.affine_select(
                        D[:, :],
                        par[:, i : i + 1].broadcast_to([128, 128]),
                        pattern=[[1, 128]],
                        compare_op=mybir.AluOpType.is_equal,
                        fill=0.0, base=0, channel_multiplier=-1,
                    )
                    diags.append(D)

                zs = []
                for pole in range(2):
                    wcol = par[:, pole : pole + 1]
                    lncol = par[:, 5 + pole : 6 + pole]
                    wtile = wp.tile([128, HEAD], f32)
                    # w^j = exp(j * ln w)
                    nc.scalar.activation(wtile[:, :], iota[:, :],
                                         mybir.ActivationFunctionType.Exp,
                                         scale=lncol)
                    scr = scrp.tile([128, HEAD], f32)
                    h0 = h0p.tile([128, 1], f32)
                    # h0 = sum_j w^j st[j]
                    nc.vector.scalar_tensor_tensor(
                        scr[:, :], st[:, 0:HEAD], 1.0, wtile[:, :],
                        bypass, mult, accum_out=h0[:, :],
                    )
                    # z[m] = w z[m+1] + x[m+1], m = L-2 .. 0;  z[L-1] = h0.
                    # The very last scan is split so its high half's
                    # assembly can overlap the low half's scan.
                    z = zp.tile([128, L], f32)
                    if t == NTILES - 1 and pole == 1:
                        nc.vector.tensor_tensor_scan(
                            _rev(z[:, HL : L - 1]),
                            wcol.broadcast_to([128, HL - 1]),
                            _rev(st[:, HEAD + HL + 1 : HEAD + L]),
                            h0[:, :], mult, add,
                        )
                        nc.vector.tensor_tensor_scan(
                            _rev(z[:, 0:HL]),
                            wcol.broadcast_to([128, HL]),
                            _rev(st[:, HEAD + 1 : HEAD + HL + 1]),
                            z[:, HL : HL + 1], mult, add,
                        )
                    else:
                        nc.vector.tensor_tensor_scan(
                            _rev(z[:, 0 : L - 1]),
                            wcol.broadcast_to([128, L - 1]),
                            _rev(st[:, HEAD + 1 : HEAD + L]),
                            h0[:, :], mult, add,
                        )
                    nc.scalar.copy(z[:, L - 1 : L], h0[:, :])
                    zs.append(z)

                # y = n0*x + r1*z1 + r2*z2 on TensorE via diag matmuls,
                # accumulated in PSUM; ScalarE copies out.  Term-major
                # emission: the z2 matmuls are the only post-scan PE work.
                acc0 = psp.tile([128, HL], f32)
                acc1 = psp.tile([128, HL], f32)
                accs = [acc0, acc1]
                for term, src, off in (
                    (0, st, HEAD), (1, zs[0], 0), (2, zs[1], 0),
                ):
                    horder = (1, 0) if term == 2 else (0, 1)
                    for h in horder:
                        for q in range(2):
                            c0 = h * HL + q * 512
                            nc.tensor.matmul(
                                accs[h][:, q * 512 : (q + 1) * 512],
                                diags[term][:, :],
                                src[:, off + c0 : off + c0 + 512],
                                start=(term == 0), stop=(term == 2),
                            )
                for h in (1, 0):
                    for q in range(2):
                        ysb = yp.tile([128, 512], f32)
                        nc.scalar.copy(ysb[:, :],
                                       accs[h][:, q * 512 : (q + 1) * 512])
                        c0 = h * HL + q * 512
                        nc.sync.dma_start(y_out[rows, c0 : c0 + 512],
                                          ysb[:, :])

    nc.compile()
    return nc


_NC_CACHE = None


def _get_nc():
    global _NC_CACHE
    if _NC_CACHE is None:
        _NC_CACHE = _build_nc()
    return _NC_CACHE


def kernel(x, cond, state, alpha_rg, alpha_r1, alpha_c1, alpha_c2,
           cond_w, cond_b):
    global LAST_RESULT
    x = np.ascontiguousarray(np.asarray(x, np.float32)[:, :, 0])      # [B, L]
    state_head = np.ascontiguousarray(
        np.asarray(state, np.float32)[:, L : L + HEAD, 0])            # [B, HEAD]

    w1, w2, r1, r2, n0 = _derive_filter_params(
        np.asarray(cond), np.asarray(alpha_rg), np.asarray(alpha_r1),
        np.asarray(alpha_c1), np.asarray(alpha_c2), np.asarray(cond_w),
        np.asarray(cond_b),
    )
    fparams = np.zeros((B, 8), np.float32)
    fparams[:, 0] = w1
    fparams[:, 1] = w2
    fparams[:, 2] = r1
    fparams[:, 3] = r2
    fparams[:, 4] = n0
    fparams[:, 5] = np.log(w1)
    fparams[:, 6] = np.log(w2)

    nc = _get_nc()
    in_maps = []
    for c in range(N_CORES):
        rows = slice(c * BPC, (c + 1) * BPC)
        in_maps.append({
            "x": x[rows],
            "state_head": state_head[rows],
            "fparams": fparams[rows],
        })

    res = run_bass_kernel_spmd(
        nc, in_maps, core_ids=list(range(N_CORES)), trace=TRACE
    )
    LAST_RESULT = res
    out = np.concatenate([r["y"] for r in res.results], axis=0)
    return out.reshape(B, L, 1).astype(np.float32)


# revision 14
# speedup vs baseline: 1.4439x; 1.0338x over previous
"""Trainium2 Bass kernel for nn_PreampGainLayer.

The reference computes, per batch row b:
    st = concat(state[b, L:], x[b])                    # length N=4096
    out[b] = irfft(rfft(st) * h_b)[-L:]                # circular filter
where h_b = num_b(w)/den_b(w) evaluated at w = e^{-i pi k/2048} is a biquad
transfer function whose denominator has two REAL roots w1, w2 with |w| < 1
(RC circuit, bilinear transform).  Partial fractions give

    H_b(w) = n0_b + r1_b/(w - w1_b) + r2_b/(w - w2_b)

and the circular convolution with each 1/(w - wi) kernel is an anti-causal
one-pole IIR: z_i[m] = sum_{s>=0} wi^s st[(m+1+s) mod N]  (wi^N ~ 1e-38 -> 0).

Device algorithm per batch (batch rows on SBUF partitions):
    h0_i = sum_{j<1024} wi^j st[j]          (wi^1024 < 2e-9: tail truncated)
         = accum_out of one fused multiply, weights wi^j = Exp(j*ln wi)
           built on ScalarE
    tail-scan (backward over x half, initial=h0_i): exact circular z_i
           via the DVE tensor_tensor_scan recurrence state=w*state+data
    y = n0*x + r1*z1 + r2*z2                (scalar_tensor_tensor FMAs)

The O(B) scalar parameter derivation (2x2 algebra + root finding) runs on
host in float64; all per-sample work runs on the NeuronCores, data-parallel
over batch (8 cores x 256 rows).
"""

import numpy as np

import concourse.bass as bass
import concourse.bacc as bacc
import concourse.tile as tile
import concourse.mybir as mybir
from concourse.bass_utils import run_bass_kernel_spmd

N_CORES = 8
B = 2048
L = 2048          # x length == output length
HEAD = 768        # retained prefix of the state half (w^768 < 3e-7)
BPC = B // N_CORES
NTILES = BPC // 128
HL = L // 2       # half of the output, for pipelining chunks
XB = HEAD + 8     # column offset of x within the st tile

SR = 44100
RG0, R10, C10, C20 = 1.0e6, 4.7e5, 3.3e-9, 1.0e-9

_Nu = np.array([[1.0, 0.0, 0.0]])
_Nx = np.array([[1.0, -1.0, 0.0], [0.0, 0.0, 1.0]])
_Nr = np.array([[0.0, 1.0, 0.0]])
_Nv = np.array([[0.0, 1.0, -1.0], [0.0, 0.0, 1.0]])
_No = np.array([[0.0, 0.0, 1.0]])

TRACE = False
LAST_RESULT = None


def _sigmoid(x):
    return 1.0 / (1.0 + np.exp(-x))


def _derive_filter_params(cond, alpha_rg, alpha_r1, alpha_c1, alpha_c2,
                          cond_w, cond_b):
    """Float64 host derivation of per-batch (w1, w2, r1, r2, n0)."""
    T = 1.0 / SR
    cond = np.asarray(cond, np.float64)
    Bn = cond.shape[0]

    RG = (0.9 + _sigmoid(float(alpha_rg[0])) * 0.2) * RG0
    R1 = (0.99 + _sigmoid(float(alpha_r1[0])) * 0.02) * R10
    C1 = (0.9 + _sigmoid(float(alpha_c1[0])) * 0.2) * C10
    C2 = (0.9 + _sigmoid(float(alpha_c2[0])) * 0.2) * C20

    Gr = np.array([[1.0 / R1]])
    Gx = np.diag([2.0 * C1 / T, 2.0 * C2 / T])

    Nvp = np.concatenate([_Nv, np.zeros((2, 1))], axis=1)
    Nxp = np.concatenate([_Nx, np.zeros((2, 1))], axis=1)
    Nop = np.concatenate([_No, np.zeros((1, 1))], axis=1)
    Nup = np.concatenate([np.zeros((3, 1)), np.eye(1)], axis=0)

    top = np.concatenate([_Nr.T @ Gr @ _Nr + _Nx.T @ Gx @ _Nx, _Nu.T], axis=1)
    bot = np.concatenate([_Nu, np.zeros((1, 1))], axis=1)
    So_inv = np.linalg.inv(np.concatenate([top, bot], axis=0))

    Q = Nvp @ So_inv @ Nvp.T
    Ux = Nxp @ So_inv @ Nvp.T
    Uo = Nop @ So_inv @ Nvp.T
    Uu = Nup.T @ So_inv @ Nvp.T
    ZGx = 2.0 * Gx
    Ao = ZGx @ Nxp @ So_inv @ Nxp.T - np.eye(2)
    Bo = ZGx @ Nxp @ So_inv @ Nup
    Do = Nop @ So_inv @ Nxp.T
    Eo = Nop @ So_inv @ Nup
    ZGxUx = ZGx @ Ux

    pot = _sigmoid(cond[:, 0] * float(cond_w[0]) + float(cond_b[0]))
    p = np.clip((np.power(10.0, pot) - 1.0) / 9.0, 1e-4, 1.0 - 1e-4)

    M00 = (1.0 - p) * RG + Q[0, 0]
    M01 = np.full(Bn, Q[0, 1])
    M10 = np.full(Bn, Q[1, 0])
    M11 = p * RG + Q[1, 1]
    detM = M00 * M11 - M01 * M10
    I00, I01 = M11 / detM, -M01 / detM
    I10, I11 = -M10 / detM, M00 / detM

    def sandwich(Lm, Rm):
        out = np.empty((Bn, Lm.shape[0], Rm.shape[1]))
        for i in range(Lm.shape[0]):
            for j in range(Rm.shape[1]):
                out[:, i, j] = (
                    Lm[i, 0] * (I00 * Rm[0, j] + I01 * Rm[1, j])
                    + Lm[i, 1] * (I10 * Rm[0, j] + I11 * Rm[1, j])
                )
        return out

    A = Ao[None] - sandwich(ZGxUx, Ux.T)
    Bm = Bo[None] - sandwich(ZGxUx, Uu.T)
    Dm = Do[None] - sandwich(Uo, Ux.T)
    Em = Eo[None] - sandwich(Uo, Uu.T)

    tr = A[:, 0, 0] + A[:, 1, 1]
    det = A[:, 0, 0] * A[:, 1, 1] - A[:, 0, 1] * A[:, 1, 0]
    dd1, dd2 = -tr, det

    M2 = A - Bm @ Dm
    tr2 = M2[:, 0, 0] + M2[:, 1, 1]
    det2 = M2[:, 0, 0] * M2[:, 1, 1] - M2[:, 0, 1] * M2[:, 1, 0]
    e = Em[:, 0, 0] - 1.0
    n0 = 1.0 + e
    n1 = -tr2 + e * dd1
    n2 = det2 + e * dd2

    a = n1 - n0 * dd1
    b = n2 - n0 * dd2
    disc = dd1 * dd1 - 4.0 * dd2
    if np.any(disc <= 0):
        raise ValueError("complex poles: real-pole fast path invalid")
    sq = np.sqrt(disc)
    w1 = 0.5 * (-dd1 + sq)
    w2 = 0.5 * (-dd1 - sq)
    r1 = (a * w1 + b) / (w1 - w2)
    r2 = (a * w2 + b) / (w2 - w1)
    return w1, w2, r1, r2, n0


def _rev(ap):
    """Reverse the innermost free dim of an AP (unit-stride dims only)."""
    step, cnt = ap.ap[-1]
    assert step == 1, ap.ap
    return bass.AP(
        tensor=ap.tensor,
        offset=ap.offset + cnt - 1,
        ap=[list(p) for p in ap.ap[:-1]] + [[-1, cnt]],
    )


def _build_nc():
    f32 = mybir.dt.float32
    mult, add = mybir.AluOpType.mult, mybir.AluOpType.add
    bypass = mybir.AluOpType.bypass

    nc = bacc.Bacc("TRN2", target_bir_lowering=False, debug=False)
    x_in = nc.dram_tensor("x", [BPC, L], f32, kind="ExternalInput")
    # state head columns [0, HEAD) plus packed per-batch filter params
    # in columns [HEAD, HEAD+8): w1, w2, r1, r2, n0, lnw1, lnw2, pad
    s_in = nc.dram_tensor("sp", [BPC, HEAD + 8], f32, kind="ExternalInput")
    y_out = nc.dram_tensor("y", [BPC, L], f32, kind="ExternalOutput")

    with tile.TileContext(nc) as tc:
        with (
            tc.tile_pool(name="sig", bufs=2) as sig,
            tc.tile_pool(name="par", bufs=2) as parp,
            tc.tile_pool(name="wgt", bufs=4) as wp,
            tc.tile_pool(name="scr", bufs=2) as scrp,
            tc.tile_pool(name="h0", bufs=4) as h0p,
            tc.tile_pool(name="z", bufs=4) as zp,
            tc.tile_pool(name="dg", bufs=6) as dgp,
            tc.tile_pool(name="y", bufs=4) as yp,
            tc.tile_pool(name="ps", bufs=2, space="PSUM") as psp,
            tc.tile_pool(name="const", bufs=1) as cst,
        ):
            iota = cst.tile([128, HEAD], f32)
            nc.gpsimd.iota(iota[:, :], pattern=[[1, HEAD]], base=0,
                           channel_multiplier=0,
                           allow_small_or_imprecise_dtypes=True)
            # Warm the ACT exp table immediately so the real Exp calls
            # don't pay the ~1.3us ACT_TABLE_LOAD on the critical path.
            warm = cst.tile([128, 1], f32)
            nc.gpsimd.memset(warm[:, :], 0.0)
            nc.scalar.activation(warm[:, :], warm[:, :],
                                 mybir.ActivationFunctionType.Exp)

            for t in range(NTILES):
                rows = slice(t * 128, (t + 1) * 128)
                # st layout: cols [0, HEAD) = state head, [HEAD, HEAD+8) =
                # params, [HEAD+8, HEAD+8+L) = x.  Tile 0 loads on the Sync
                # DGE ring (x first - the scans gate on it); tile 1 loads on
                # the ScalarE ring so the rings work in parallel.
                st = sig.tile([128, HEAD + 8 + L], f32)
                par = st[:, HEAD : HEAD + 8]
                dge = nc.sync if t == 0 else nc.scalar
                dge.dma_start(st[:, 0 : HEAD + 8], s_in[rows, :])
                dge.dma_start(st[:, HEAD + 8 : HEAD + 8 + L], x_in[rows, :])

                # diag(n0), diag(r1), diag(r2) for the TensorE assembly
                diags = []
                for i in (4, 2, 3):
                    D = dgp.tile([128, 128], f32)
                    nc.gpsimd.affine_select(
                        D[:, :],
                        par[:, i : i + 1].broadcast_to([128, 128]),
                        pattern=[[1, 128]],
                        compare_op=mybir.AluOpType.is_equal,
                        fill=0.0, base=0, channel_multiplier=-1,
                    )
                    diags.append(D)

                zs = []
                for pole in range(2):
                    wcol = par[:, pole : pole + 1]
                    lncol = par[:, 5 + pole : 6 + pole]
                    wtile = wp.tile([128, HEAD], f32)
                    # w^j = exp(j * ln w)
                    nc.scalar.activation(wtile[:, :], iota[:, :],
                                         mybir.ActivationFunctionType.Exp,
                                         scale=lncol)
                    scr = scrp.tile([128, HEAD], f32)
                    h0 = h0p.tile([128, 1], f32)
                    # h0 = sum_j w^j st[j]
                    nc.vector.scalar_tensor_tensor(
                        scr[:, :], st[:, 0:HEAD], 1.0, wtile[:, :],
                        bypass, mult, accum_out=h0[:, :],
                    )
                    # z[m] = w z[m+1] + x[m+1], m = L-2 .. 0;  z[L-1] = h0.
                    # The very last scan is split so its high half's
                    # assembly can overlap the low half's scan.
                    z = zp.tile([128, L], f32)
                    if t == NTILES - 1 and pole == 1:
                        nc.vector.tensor_tensor_scan(
                            _rev(z[:, HL : L - 1]),
                            wcol.broadcast_to([128, HL - 1]),
                            _rev(st[:, XB + HL + 1 : XB + L]),
                            h0[:, :], mult, add,
                        )
                        nc.vector.tensor_tensor_scan(
                            _rev(z[:, 0:HL]),
                            wcol.broadcast_to([128, HL]),
                            _rev(st[:, XB + 1 : XB + HL + 1]),
                            z[:, HL : HL + 1], mult, add,
                        )
                    else:
                        nc.vector.tensor_tensor_scan(
                            _rev(z[:, 0 : L - 1]),
                            wcol.broadcast_to([128, L - 1]),
                            _rev(st[:, XB + 1 : XB + L]),
                            h0[:, :], mult, add,
                        )
                    nc.scalar.copy(z[:, L - 1 : L], h0[:, :])
                    zs.append(z)

                # y = n0*x + r1*z1 + r2*z2 on TensorE via diag matmuls,
                # accumulated in PSUM; ScalarE copies out.  Term-major
                # emission: the z2 matmuls are the only post-scan PE work.
                acc0 = psp.tile([128, HL], f32)
                acc1 = psp.tile([128, HL], f32)
                accs = [acc0, acc1]
                for term, src, off in (
                    (0, st, XB), (1, zs[0], 0), (2, zs[1], 0),
                ):
                    horder = (1, 0) if term == 2 else (0, 1)
                    for h in horder:
                        for q in range(2):
                            c0 = h * HL + q * 512
                            nc.tensor.matmul(
                                accs[h][:, q * 512 : (q + 1) * 512],
                                diags[term][:, :],
                                src[:, off + c0 : off + c0 + 512],
                                start=(term == 0), stop=(term == 2),
                            )
                for h in (1, 0):
                    for q in range(2):
                        ysb = yp.tile([128, 512], f32)
                        nc.scalar.copy(ysb[:, :],
                                       accs[h][:, q * 512 : (q + 1) * 512])
                        c0 = h * HL + q * 512
                        nc.sync.dma_start(y_out[rows, c0 : c0 + 512],
                                          ysb[:, :])

    nc.compile()
    return nc


_NC_CACHE = None


def _get_nc():
    global _NC_CACHE
    if _NC_CACHE is None:
        _NC_CACHE = _build_nc()
    return _NC_CACHE


def kernel(x, cond, state, alpha_rg, alpha_r1, alpha_c1, alpha_c2,
           cond_w, cond_b):
    global LAST_RESULT
    x = np.ascontiguousarray(np.asarray(x, np.float32)[:, :, 0])      # [B, L]
    state_head = np.ascontiguousarray(
        np.asarray(state, np.float32)[:, L : L + HEAD, 0])            # [B, HEAD]

    w1, w2, r1, r2, n0 = _derive_filter_params(
        np.asarray(cond), np.asarray(alpha_rg), np.asarray(alpha_r1),
        np.asarray(alpha_c1), np.asarray(alpha_c2), np.asarray(cond_w),
        np.asarray(cond_b),
    )
    fparams = np.zeros((B, 8), np.float32)
    fparams[:, 0] = w1
    fparams[:, 1] = w2
    fparams[:, 2] = r1
    fparams[:, 3] = r2
    fparams[:, 4] = n0
    fparams[:, 5] = np.log(w1)
    fparams[:, 6] = np.log(w2)
    sp = np.ascontiguousarray(
        np.concatenate([state_head, fparams], axis=1))        # [B, HEAD+8]

    nc = _get_nc()
    in_maps = []
    for c in range(N_CORES):
        rows = slice(c * BPC, (c + 1) * BPC)
        in_maps.append({
            "x": x[rows],
            "sp": sp[rows],
        })

    res = run_bass_kernel_spmd(
        nc, in_maps, core_ids=list(range(N_CORES)), trace=TRACE
    )
    LAST_RESULT = res
    out = np.concatenate([r["y"] for r in res.results], axis=0)
    return out.reshape(B, L, 1).astype(np.float32)
